# revision 20
# baseline (speedup 1.0000x reference)
"""Trainium2 Bass kernel for the show-attend-tell style attention module.

  att_h   = h @ W_h2att.T + b_h2att                      # [B, H]
  dot     = tanh(p_att_feats + att_h[:, None, :])        # [B, S, H]
  scores  = dot @ w_alpha + b_alpha                      # [B, S]
  weight  = softmax(scores) * mask, renormalized         # [B, S]
  att_res = sum_s weight[:, s] * att_feats[:, s, :]      # [B, D]

B=256, S=196, D=2048, H=512.  Data-parallel over 8 NeuronCores (32
batches per core); params replicated.  b_alpha cancels inside softmax
and is ignored.  The mask renorm is fused into the softmax denominator:
weight = exp(s - max) * mask / sum(exp(s - max) * mask), which equals
the reference's softmax -> mask -> renormalize chain exactly (the first
softmax's denominator cancels).

Memory-bound problem: the kernel streams att_feats (51.4 MB/core) and
p_att_feats (12.8 MB/core) exactly once.  The weighted sum runs on the
PE in float32r (full-rate fp32 matmul mode).

PE matmul outputs must start at PSUM partition 0/32/64/96, so batches
are processed in quartets: batch q lands at partition offset 32*q of
shared score / result PSUM tiles.
"""

import sys

if "/opt/trn_rl_repo" not in sys.path:
    sys.path.insert(0, "/opt/trn_rl_repo")

from contextlib import ExitStack

import numpy as np

import concourse.bacc as bacc
import concourse.tile as tile
from concourse import mybir
from concourse.bass_utils import run_bass_kernel_spmd
from concourse.masks import make_identity

# Problem dims (hardcoded per the harness contract).
B, S, D, H = 256, 196, 2048, 512
P = 128          # partitions
HC = H // P      # 4 h-chunks
DC = D // P      # 16 d-chunks
NCH = D // 512   # 4 output column chunks of 512
S0 = 128         # first s-chunk rows
S1 = S - S0      # second s-chunk rows (68)
G = 4            # batches per quartet (PSUM partition groups)
N_CORES = 8
BS = B // N_CORES  # 32 batches per core

FP32 = mybir.dt.float32
FP32R = mybir.dt.float32r
AX = mybir.AxisListType
AF = mybir.ActivationFunctionType


def build_program(bs=BS, score_dtype=FP32):
    """Build the single-core Bass/Tile program (SPMD across cores)."""
    nc = bacc.Bacc("TRN2", target_bir_lowering=False, debug=False)

    assert bs % G == 0
    ngroups = bs // G

    feats = nc.dram_tensor("feats", [bs, S, D], FP32R, kind="ExternalInput").ap()
    pT = nc.dram_tensor("pT", [bs, P, HC * S], FP32, kind="ExternalInput").ap()
    hT = nc.dram_tensor("hT", [P, DC * bs], FP32, kind="ExternalInput").ap()
    WT = nc.dram_tensor("WT", [P, DC * H], FP32, kind="ExternalInput").ap()
    wal = nc.dram_tensor("walpha", [P, HC], score_dtype, kind="ExternalInput").ap()
    bh = nc.dram_tensor("bh", [1, H], FP32, kind="ExternalInput").ap()
    masks = nc.dram_tensor("masks", [bs, S], FP32, kind="ExternalInput").ap()
    out = nc.dram_tensor("out", [bs, D], FP32, kind="ExternalOutput").ap()

    with tile.TileContext(nc) as tc, ExitStack() as ctx:
        singles = ctx.enter_context(tc.tile_pool(name="singles", bufs=1))
        ppool = ctx.enter_context(tc.tile_pool(name="ppool", bufs=4))
        dpool = ctx.enter_context(tc.tile_pool(name="dpool", bufs=2))
        fpool = ctx.enter_context(tc.tile_pool(name="fpool", bufs=8))
        gpool = ctx.enter_context(tc.tile_pool(name="gpool", bufs=2))
        ps_att = ctx.enter_context(tc.tile_pool(name="ps_att", bufs=1, space="PSUM"))
        ps_sc = ctx.enter_context(tc.tile_pool(name="ps_sc", bufs=2, space="PSUM"))
        ps_wt = ctx.enter_context(tc.tile_pool(name="ps_wt", bufs=1, space="PSUM"))
        ps_res = ctx.enter_context(tc.tile_pool(name="ps_res", bufs=2, space="PSUM"))

        # ---- constants / params ----
        # W^T is loaded in 16 d-chunks so the att_h matmuls can start as soon
        # as the first chunk lands instead of waiting for the full 4 MB.
        ht_sb = singles.tile([P, DC * bs], FP32)
        nc.gpsimd.dma_start(out=ht_sb, in_=hT)
        wt_sb = singles.tile([P, DC * H], FP32)
        for dc in range(DC):
            nc.gpsimd.dma_start(
                out=wt_sb[:, dc * H : (dc + 1) * H], in_=WT[:, dc * H : (dc + 1) * H]
            )
        wal_sb = singles.tile([P, HC], score_dtype)
        nc.gpsimd.dma_start(out=wal_sb, in_=wal)
        bh_sb = singles.tile([1, H], FP32)
        nc.gpsimd.dma_start(out=bh_sb, in_=bh)
        ones_sb = singles.tile([1, bs], FP32)
        nc.vector.memset(ones_sb, 1.0)
        ident = singles.tile([P, P], FP32)
        make_identity(nc, ident)

        # ---- att_h^T = W @ h^T + b  ->  [P, HC, bs] (h-chunk on partitions) ----
        # PSUM tiles are padded to whole 2 KiB banks (512 f32 / partition).
        atth_ps_full = ps_att.tile([P, HC, P], FP32)
        atth_ps = atth_ps_full[:, :, 0:bs]
        for dc in range(DC):
            for hc in range(HC):
                nc.tensor.matmul(
                    atth_ps[:, hc, :],
                    lhsT=wt_sb[:, dc * H + hc * P : dc * H + (hc + 1) * P],
                    rhs=ht_sb[:, dc * bs : (dc + 1) * bs],
                    start=(dc == 0),
                    stop=False,
                )
        for hc in range(HC):
            # bias: rank-1 update ones^T x b_h2att
            nc.tensor.matmul(
                atth_ps[:, hc, :],
                lhsT=bh_sb[:, hc * P : (hc + 1) * P],
                rhs=ones_sb,
                start=False,
                stop=True,
            )
        atth_sb = singles.tile([P, HC, bs], FP32)
        nc.vector.tensor_copy(out=atth_sb, in_=atth_ps)

        def phase_a(gi):
            """Scores + masked softmax + weight transpose for quartet gi.

            Batch q sits at partition offset 32*q; unused rows are zeroed so
            the batched softmax stays NaN-free.  Returns wtT_sb.
            """
            sc_ps_full = ps_sc.tile([P, 512], FP32)
            sc_ps = sc_ps_full[:, 0:S]
            nc.vector.memset(sc_ps, 0.0)
            msk = gpool.tile([P, S], FP32)
            nc.vector.memset(msk, 1.0)
            for q in range(G):
                b = gi * G + q
                poff = 32 * q
                p_sb = ppool.tile([P, HC * S], FP32)
                nc.scalar.dma_start(out=p_sb, in_=pT[b])
                dot_sb = dpool.tile([P, HC * S], score_dtype)
                for hc in range(HC):
                    nc.scalar.activation(
                        out=dot_sb[:, hc * S : (hc + 1) * S],
                        in_=p_sb[:, hc * S : (hc + 1) * S],
                        func=AF.Tanh,
                        bias=atth_sb[:, hc, b : b + 1],
                        scale=1.0,
                    )
                for hc in range(HC):
                    nc.tensor.matmul(
                        sc_ps[poff : poff + 1, :],
                        lhsT=wal_sb[:, hc : hc + 1],
                        rhs=dot_sb[:, hc * S : (hc + 1) * S],
                        start=(hc == 0),
                        stop=(hc == HC - 1),
                        tile_position=(0, poff),
                    )
                nc.gpsimd.dma_start(
                    out=msk[poff : poff + 1, :], in_=masks[b : b + 1, :]
                )

            # batched masked softmax over s for the quartet
            mx = gpool.tile([P, 1], FP32)
            nc.vector.reduce_max(mx, sc_ps, axis=AX.X)
            nm = gpool.tile([P, 1], FP32)
            nc.vector.tensor_scalar_mul(nm, mx, -1.0)
            e_sb = gpool.tile([P, S], FP32)
            nc.scalar.activation(out=e_sb, in_=sc_ps, func=AF.Exp, bias=nm, scale=1.0)
            em = gpool.tile([P, S], FP32)
            nc.vector.tensor_mul(em, e_sb, msk)
            zz = gpool.tile([P, 1], FP32)
            nc.vector.reduce_sum(zz, em, axis=AX.X)
            rz = gpool.tile([P, 1], FP32)
            nc.vector.reciprocal(rz, zz)
            wgt = gpool.tile([P, S], FP32)
            nc.vector.tensor_scalar_mul(wgt, em, rz)

            # transpose weights -> [S, P] (batch q in column 32*q)
            wtT_ps_full = ps_wt.tile([P, 2, 256], FP32)
            wtT_ps = wtT_ps_full[:, :, 0:P]
            nc.tensor.transpose(wtT_ps[:, 0, :], wgt[:, 0:S0], ident)
            nc.tensor.transpose(wtT_ps[0:S1, 1, :], wgt[:, S0:S], ident)
            wtT_sb = gpool.tile([P, 2, P], FP32R)
            nc.vector.tensor_copy(out=wtT_sb[:, 0, :], in_=wtT_ps[:, 0, :])
            nc.vector.tensor_copy(out=wtT_sb[0:S1, 1, :], in_=wtT_ps[0:S1, 1, :])
            return wtT_sb

        def phase_b(gi, wtT_sb):
            """att_res rows for quartet gi via float32r matmuls.

            float32r matmuls may only write PSUM partition base 0, so each
            batch accumulates into [1, 1024] half-rows (2 banks, double
            buffered) that are copied out while the next half runs.
            """
            for q in range(G):
                b = gi * G + q
                poff = 32 * q
                f0 = fpool.tile([P, D], FP32R)
                nc.sync.dma_start(out=f0, in_=feats[b, 0:S0, :])
                f1 = fpool.tile([P, D], FP32R)
                nc.sync.dma_start(out=f1[0:S1, :], in_=feats[b, S0:S, :])
                row_sb = gpool.tile([1, NCH, 512], FP32, tag="row_sb")
                for half in range(2):
                    res_ps = ps_res.tile([1, 2, 512], FP32)
                    for c2 in range(2):
                        cc = half * 2 + c2
                        nc.tensor.matmul(
                            res_ps[0:1, c2, :],
                            lhsT=wtT_sb[:, 0, poff : poff + 1],
                            rhs=f0[:, cc * 512 : (cc + 1) * 512],
                            start=True,
                            stop=False,
                        )
                        nc.tensor.matmul(
                            res_ps[0:1, c2, :],
                            lhsT=wtT_sb[0:S1, 1, poff : poff + 1],
                            rhs=f1[0:S1, cc * 512 : (cc + 1) * 512],
                            start=False,
                            stop=True,
                        )
                    # alternate copy engines so copies never pace the PE
                    if half == 0:
                        nc.vector.tensor_copy(
                            out=row_sb[0:1, 0:2, :], in_=res_ps
                        )
                    else:
                        nc.scalar.copy(out=row_sb[0:1, 2:4, :], in_=res_ps)
                nc.gpsimd.dma_start(out=out[b : b + 1, :], in_=row_sb)

        # Software pipeline: phase A of group g+1 is emitted before phase B of
        # group g, so the softmax/transpose latency of g+1 hides under g's
        # result matmuls on the PE.
        prev = None
        for gi in range(ngroups):
            wtT = phase_a(gi)
            if prev is not None:
                phase_b(gi - 1, prev)
            prev = wtT
        phase_b(ngroups - 1, prev)

    nc.compile()
    return nc


def host_prepare(inputs, bs=BS):
    """Pre-layout full inputs into per-core in_maps (host-side, untimed)."""
    h = np.ascontiguousarray(np.asarray(inputs["h"], dtype=np.float32))
    att_feats = np.asarray(inputs["att_feats"], dtype=np.float32)
    p = np.asarray(inputs["p_att_feats"], dtype=np.float32)
    att_masks = np.asarray(inputs["att_masks"], dtype=np.float32)
    W = np.asarray(inputs["W_h2att"], dtype=np.float32)
    b_h2att = np.asarray(inputs["b_h2att"], dtype=np.float32)
    w_alpha = np.asarray(inputs["w_alpha"], dtype=np.float32)

    n_cores = h.shape[0] // bs

    # [P, DC*H]: WT[p, dc*H + h] = W^T[dc*P + p, h] = W[h, dc*P + p]
    WT = np.ascontiguousarray(
        W.T.reshape(DC, P, H).transpose(1, 0, 2).reshape(P, DC * H)
    )
    # [P, HC]: wal[p, hc] = w_alpha[hc*P + p]
    wal = np.ascontiguousarray(w_alpha.reshape(HC, P).T)
    bh = np.ascontiguousarray(b_h2att.reshape(1, H))
    # [B, P, HC*S]: pT[b, p, hc*S + s] = p[b, s, hc*P + p]
    pT = np.ascontiguousarray(
        p.reshape(-1, S, HC, P).transpose(0, 3, 2, 1).reshape(-1, P, HC * S)
    )

    in_maps = []
    for c in range(n_cores):
        b0 = c * bs
        h_sh = h[b0 : b0 + bs]  # [bs, D]
        # [P, DC*bs]: hT[p, dc*bs + b] = h[b, dc*P + p]
        hT = np.ascontiguousarray(
            h_sh.T.reshape(DC, P, bs).transpose(1, 0, 2).reshape(P, DC * bs)
        )
        in_maps.append(
            {
                "feats": np.ascontiguousarray(att_feats[b0 : b0 + bs]),
                "pT": np.ascontiguousarray(pT[b0 : b0 + bs]),
                "hT": hT,
                "WT": WT,
                "walpha": wal,
                "bh": bh,
                "masks": np.ascontiguousarray(att_masks[b0 : b0 + bs]),
            }
        )
    return in_maps


_PROGRAM = None


def _get_program():
    global _PROGRAM
    if _PROGRAM is None:
        _PROGRAM = build_program()
    return _PROGRAM


def run(inputs, trace=False):
    nc = _get_program()
    in_maps = host_prepare(inputs)
    res = run_bass_kernel_spmd(nc, in_maps, list(range(N_CORES)), trace=trace)
    out = np.concatenate([r["out"] for r in res.results], axis=0)
    return out, res


def kernel(**inputs) -> np.ndarray:
    out, _ = run(inputs, trace=False)
    return out


def bench(inputs, iters=20, warmup=3):
    """Time device execution: inputs staged on device once, then `iters`
    back-to-back pipelined executions (hides per-call dispatch latency).
    Returns (best_single_call_s, pipelined_avg_s, out).

    No NTFF profiling is available in this container (antenv.axon_hooks is
    absent), so this wall-clock path is the hardware timing source.
    """
    import time

    import jax
    from jax.experimental.shard_map import shard_map
    from jax.sharding import Mesh, NamedSharding, PartitionSpec

    from concourse import bass2jax, mybir
    from concourse.bass2jax import _bass_exec_p, partition_id_tensor

    nc = _get_program()
    in_maps = host_prepare(inputs)
    n_cores = N_CORES
    bass2jax.install_neuronx_cc_hook()

    partition_name = (
        nc.partition_id_tensor.name if nc.partition_id_tensor else None
    )
    in_names, out_names, out_avals = [], [], []
    for alloc in nc.m.functions[0].allocations:
        if not isinstance(alloc, mybir.MemoryLocationSet):
            continue
        name = alloc.memorylocations[0].name
        if alloc.kind == "ExternalInput":
            if name != partition_name:
                in_names.append(name)
        elif alloc.kind == "ExternalOutput":
            out_names.append(name)
            out_avals.append(
                jax.core.ShapedArray(
                    tuple(alloc.tensor_shape), mybir.dt.np(alloc.dtype)
                )
            )
    n_params = len(in_names)
    all_in_names = list(in_names) + list(out_names)
    if partition_name is not None:
        all_in_names.append(partition_name)

    def _body(*args):
        operands = list(args)
        if partition_name is not None:
            operands.append(partition_id_tensor())
        outs = _bass_exec_p.bind(
            *operands,
            out_avals=tuple(out_avals),
            in_names=tuple(all_in_names),
            out_names=tuple(out_names),
            lowering_input_output_aliases=(),
            sim_require_finite=True,
            sim_require_nnan=True,
            nc=nc,
        )
        return tuple(outs)

    devices = jax.devices()[:n_cores]
    mesh = Mesh(np.asarray(devices), ("core",))
    n_outs = len(out_avals)
    in_specs = (PartitionSpec("core"),) * (n_params + n_outs)
    out_specs = (PartitionSpec("core"),) * n_outs
    donate = tuple(range(n_params, n_params + n_outs))
    sharded = jax.jit(
        shard_map(
            _body, mesh=mesh, in_specs=in_specs, out_specs=out_specs,
            check_rep=False,
        ),
        donate_argnums=donate,
        keep_unused=True,
    )
    sh = NamedSharding(mesh, PartitionSpec("core"))
    concat_in = [
        jax.device_put(
            np.concatenate([in_maps[c][nm] for c in range(n_cores)], axis=0), sh
        )
        for nm in in_names
    ]
    zero_shapes = [
        (n_cores * a.shape[0], *a.shape[1:]) for a in out_avals
    ]
    zeros_fn = jax.jit(
        lambda: tuple(
            jax.numpy.zeros(s, a.dtype) for s, a in zip(zero_shapes, out_avals)
        ),
        out_shardings=tuple(sh for _ in out_avals),
    )

    out = None
    for _ in range(warmup):
        out = sharded(*concat_in, *zeros_fn())
        jax.block_until_ready(out)

    # single-call latency
    best = float("inf")
    for _ in range(5):
        z = zeros_fn()
        jax.block_until_ready(z)
        t0 = time.perf_counter()
        out = sharded(*concat_in, *z)
        jax.block_until_ready(out)
        best = min(best, time.perf_counter() - t0)

    # pipelined: issue all, then block
    zs = [zeros_fn() for _ in range(iters)]
    jax.block_until_ready(zs)
    t0 = time.perf_counter()
    outs = [sharded(*concat_in, *z) for z in zs]
    jax.block_until_ready(outs)
    piped = (time.perf_counter() - t0) / iters

    out_np = np.asarray(out[0]) if out is not None else None
    return best, piped, out_np
